# revision 5
# baseline (speedup 1.0000x reference)
"""Distributed Trainium2 attention kernel (8 NeuronCores).

Problem: B=2, T=2048, E=1024, H=16 causal attention with partial RoPE
(first quarter of head_dim rotates, second quarter identity), fused QKV
projection and output projection.

Sharding: tensor-parallel over heads — each core owns 2 heads (dd=128 of
the E dims) for both batch elements. Each core computes q/k/v for its
heads from the full x, runs causal attention, and produces a partial
output projection (contraction over its 128 context dims). The host sums
the 8 partial outputs.

On-core layout choices (all "T" suffixes mean head-dim-on-partitions):
  - qT/kT [128, 4096]: rows = 2 heads x 64 dims, cols = flat b*2048+t.
  - scores computed transposed: S_T[tk, tq] = k_h^T-slice @ q_h, with the
    two heads packed into the 128x128 PE array as row groups (K=64 each).
  - softmax without max-subtraction (scores are O(few) for this data);
    P = exp(S_T/8) via ScalarE straight out of PSUM.
  - denominators: ctx matmul lhsT is [v_h | ones] (M=65); row 64 of the
    ctx PSUM accumulator is the softmax denominator.
  - normalization: reciprocal of row 64, broadcast across 64 partitions
    with a K=1 matmul against a ones row, then one DVE multiply.
  - RoPE: y = A .* q + swap(Bp .* q) where A/Bp are host-precomputed
    [128, 2048] tables and swap is a PE matmul by a permutation matrix.
  - causal: tk-tiles beyond the diagonal are skipped; diagonal tiles are
    computed with a column-sliced rhs (N = 512-128r) and masked with a
    single [128,128] triu mask.
"""

import numpy as np
import ml_dtypes

import concourse.bacc as bacc
import concourse.mybir as mybir
from concourse import tile
from concourse.bass_utils import run_bass_kernel_spmd

BF16 = mybir.dt.bfloat16
F32 = mybir.dt.float32
npbf16 = ml_dtypes.bfloat16

B, T, E, H = 2, 2048, 1024, 16
NCORES = 8
D = E // H            # 64 head dim
HLOC = H // NCORES    # 2 heads per core
DD = HLOC * D         # 128 local context dims
TT = B * T            # 4096 flat tokens
NE = E // 128         # 8 contraction tiles for projections
NTB = TT // 512       # 8 t-blocks for qkv phase
TQB = 512             # tq block size
NJ = T // TQB         # 4 tq blocks per batch
TKT = 128             # tk tile size
NTK = T // TKT        # 16 tk tiles per batch
ROPE_BASE = 10000.0
SCALE = 1.0 / np.sqrt(D)

EXP = mybir.ActivationFunctionType.Exp


def build_nc():
    nc = bacc.Bacc(None, target_bir_lowering=False, debug=False)

    xT = nc.declare_dram_parameter("xT", [E, TT], BF16, isOutput=False)
    wqT = nc.declare_dram_parameter("wqT", [E, DD], BF16, isOutput=False)
    wkT = nc.declare_dram_parameter("wkT", [E, DD], BF16, isOutput=False)
    wvT = nc.declare_dram_parameter("wvT", [E, DD], BF16, isOutput=False)
    woT = nc.declare_dram_parameter("woT", [DD, E], BF16, isOutput=False)
    ropeA = nc.declare_dram_parameter("ropeA", [DD, T], F32, isOutput=False)
    ropeBp = nc.declare_dram_parameter("ropeBp", [DD, T], F32, isOutput=False)
    perm = nc.declare_dram_parameter("perm", [DD, DD], BF16, isOutput=False)
    triu = nc.declare_dram_parameter("triu", [TKT, TKT], BF16, isOutput=False)
    out = nc.declare_dram_parameter("out", [TT, E], F32, isOutput=True)

    with tile.TileContext(nc) as tc:
        with (
            tc.tile_pool(name="const", bufs=1) as const,
            tc.tile_pool(name="big", bufs=1) as big,
            tc.tile_pool(name="xt", bufs=12) as xtp,
            tc.tile_pool(name="rope_tmp", bufs=4) as rtp,
            tc.tile_pool(name="pbuf", bufs=10) as pbuf,
            tc.tile_pool(name="norm", bufs=4) as normp,
            tc.tile_pool(name="osb", bufs=4) as osbp,
        ):
            # ---- resident constants ----
            wq_sb = const.tile([128, NE * DD], BF16, tag="wq")
            wk_sb = const.tile([128, NE * DD], BF16, tag="wk")
            wv_sb = const.tile([128, NE * DD], BF16, tag="wv")
            for e in range(NE):
                nc.sync.dma_start(out=wq_sb[:, e * DD:(e + 1) * DD],
                                  in_=wqT[e * 128:(e + 1) * 128, :])
                nc.sync.dma_start(out=wk_sb[:, e * DD:(e + 1) * DD],
                                  in_=wkT[e * 128:(e + 1) * 128, :])
                nc.sync.dma_start(out=wv_sb[:, e * DD:(e + 1) * DD],
                                  in_=wvT[e * 128:(e + 1) * 128, :])
            wo_sb = const.tile([128, E], BF16, tag="wo")
            nc.sync.dma_start(out=wo_sb[:], in_=woT[:])
            ra_sb = const.tile([128, T], F32, tag="ra")
            rb_sb = const.tile([128, T], F32, tag="rb")
            nc.sync.dma_start(out=ra_sb[:], in_=ropeA[:])
            nc.sync.dma_start(out=rb_sb[:], in_=ropeBp[:])
            perm_sb = const.tile([128, 128], BF16, tag="perm")
            nc.sync.dma_start(out=perm_sb[:], in_=perm[:])
            triu_sb = const.tile([128, 128], BF16, tag="triu")
            nc.sync.dma_start(out=triu_sb[:], in_=triu[:])
            ones_sb = const.tile([128, 64], F32, tag="ones")
            nc.vector.memset(ones_sb[:], 1.0)

            # ---- resident activations ----
            qrot = big.tile([128, TT], BF16, tag="qrot")
            krot = big.tile([128, TT], BF16, tag="krot")
            # 32 tiles of [v_h0(64) | 1 | v_h1(64) | 1]
            v_all = big.tile([128, 32 * 130], BF16, tag="vall")
            nc.vector.memset(v_all[:], 1.0)

            # ================= phase 1: qkv + rope =================
            with tc.tile_pool(name="psA", bufs=4, space="PSUM") as psA:
                for tb in range(NTB):
                    tsl = slice(tb * 512, (tb + 1) * 512)
                    rsl = slice((tb % (NTB // B)) * 512,
                                ((tb % (NTB // B)) + 1) * 512)
                    xts = []
                    for e in range(NE):
                        xt = xtp.tile([128, 512], BF16, tag="xt")
                        nc.sync.dma_start(out=xt[:],
                                          in_=xT[e * 128:(e + 1) * 128, tsl])
                        xts.append(xt)
                    for name, w_sb, dst in (("q", wq_sb, qrot), ("k", wk_sb, krot)):
                        ps = psA.tile([128, 512], F32, tag="pa")
                        for e in range(NE):
                            nc.tensor.matmul(ps[:], w_sb[:, e * DD:(e + 1) * DD],
                                             xts[e][:], start=(e == 0),
                                             stop=(e == NE - 1))
                        # rope: z = Bp .* ps ; sw = perm^T z ; dst = A .* ps + sw
                        z = rtp.tile([128, 512], BF16, tag="z")
                        nc.vector.tensor_mul(out=z[:], in0=ps[:], in1=rb_sb[:, rsl])
                        sw = psA.tile([128, 512], F32, tag="pa")
                        nc.tensor.matmul(sw[:], perm_sb[:], z[:],
                                         start=True, stop=True)
                        y0 = rtp.tile([128, 512], F32, tag="y0")
                        nc.vector.tensor_mul(out=y0[:], in0=ps[:], in1=ra_sb[:, rsl])
                        nc.vector.tensor_add(out=dst[:, tsl], in0=y0[:], in1=sw[:])
                    # v for the 4 t-subtiles of this block
                    for s in range(4):
                        vi = tb * 4 + s
                        pv = psA.tile([128, 512], F32, tag="pa")
                        for e in range(NE):
                            nc.tensor.matmul(
                                pv[:, 0:DD],
                                xts[e][:, s * 128:(s + 1) * 128],
                                wv_sb[:, e * DD:(e + 1) * DD],
                                start=(e == 0), stop=(e == NE - 1))
                        nc.vector.tensor_copy(out=v_all[:, vi * 130:vi * 130 + 64],
                                              in_=pv[:, 0:64])
                        nc.vector.tensor_copy(out=v_all[:, vi * 130 + 65:vi * 130 + 129],
                                              in_=pv[:, 64:128])

            # ================= phase 2: attention + out proj =================
            with (
                tc.tile_pool(name="psS", bufs=3, space="PSUM") as psS,
                tc.tile_pool(name="psC", bufs=2, space="PSUM") as psC,
                tc.tile_pool(name="psO", bufs=3, space="PSUM") as psO,
            ):
                for b in range(B):
                    for j in range(NJ):
                        ni = 4 * j + 4  # tk tiles for this tq block
                        qsl = slice(b * T + j * TQB, b * T + (j + 1) * TQB)
                        pc = [psC.tile([128, 512], F32, tag="pc", name=f"pc_{b}_{j}_{h}")
                              for h in range(HLOC)]
                        for i in range(ni):
                            r = i - 4 * j  # >=0 on diagonal tiles
                            off = 128 * r if r > 0 else 0
                            n = 512 - off
                            qsl_i = slice(b * T + j * TQB + off, b * T + (j + 1) * TQB)
                            ksl = slice(b * T + i * TKT, b * T + (i + 1) * TKT)
                            vi = b * NTK + i
                            for h in range(HLOC):
                                hsl = slice(h * 64, (h + 1) * 64)
                                ps = psS.tile([128, 512], F32, tag="ps")
                                nc.tensor.matmul(ps[:, 0:n], krot[hsl, ksl],
                                                 qrot[hsl, qsl_i],
                                                 start=True, stop=True)
                                p_sb = pbuf.tile([128, 512], BF16, tag="p")
                                nc.scalar.activation(p_sb[:, off:512], ps[:, 0:n],
                                                     EXP, scale=float(SCALE))
                                if r >= 0:
                                    nc.vector.tensor_mul(
                                        out=p_sb[:, off:off + 128],
                                        in0=p_sb[:, off:off + 128],
                                        in1=triu_sb[:])
                                nc.tensor.matmul(
                                    pc[h][0:65, off:512],
                                    v_all[:, vi * 130 + h * 65:vi * 130 + h * 65 + 65],
                                    p_sb[:, off:512],
                                    start=(i == 0), stop=(i == ni - 1))
                        # normalize: recip of denominators, broadcast, multiply
                        ctxn = normp.tile([128, 512], BF16, tag="ctxn")
                        pB = psO.tile([128, 512], F32, tag="po")
                        for h in range(HLOC):
                            rc = normp.tile([65, 512], F32, tag="recip")
                            nc.vector.reciprocal(out=rc[64:65, :], in_=pc[h][64:65, :])
                            nc.tensor.matmul(pB[h * 64:(h + 1) * 64, :],
                                             ones_sb[64:65, :], rc[64:65, :],
                                             start=True, stop=True)
                        bsb = normp.tile([128, 512], F32, tag="bsb")
                        nc.vector.tensor_copy(out=bsb[:], in_=pB[:])
                        for h in range(HLOC):
                            nc.vector.tensor_mul(out=ctxn[h * 64:(h + 1) * 64, :],
                                                 in0=pc[h][0:64, :],
                                                 in1=bsb[h * 64:(h + 1) * 64, :])
                        # out projection: partial over this core's 128 dims
                        for s in range(4):
                            for f in range(2):
                                po = psO.tile([128, 512], F32, tag="po")
                                nc.tensor.matmul(po[:],
                                                 ctxn[:, s * 128:(s + 1) * 128],
                                                 wo_sb[:, f * 512:(f + 1) * 512],
                                                 start=True, stop=True)
                                o_sb = osbp.tile([128, 512], F32, tag="osb")
                                nc.vector.tensor_copy(out=o_sb[:], in_=po[:])
                                row0 = b * T + j * TQB + s * 128
                                nc.sync.dma_start(
                                    out=out[row0:row0 + 128, f * 512:(f + 1) * 512],
                                    in_=o_sb[:])
    nc.compile()
    return nc


def _rope_tables():
    quarter = D // 4  # 16
    inv = np.concatenate([
        ROPE_BASE ** (-np.arange(quarter, dtype=np.float64) / quarter),
        np.zeros(quarter),
    ])  # [32]
    t = np.arange(T, dtype=np.float64)
    theta = np.outer(t, inv)  # [T, 32]
    cos = np.cos(theta)
    sin = np.sin(theta)
    A = np.zeros((DD, T), dtype=np.float32)
    Bp = np.zeros((DD, T), dtype=np.float32)
    for dd in range(DD):
        dh = dd % D
        jj = dh % (D // 2)
        A[dd] = cos[:, jj]
        Bp[dd] = sin[:, jj] if dh < D // 2 else -sin[:, jj]
    P = np.zeros((DD, DD), dtype=np.float32)
    for k in range(DD):
        dh = k % D
        partner = k + D // 2 if dh < D // 2 else k - D // 2
        P[k, partner] = 1.0
    return A, Bp, P


_CACHE = {}


def kernel(x, Wqkv, Wout):
    x = np.asarray(x, dtype=np.float32)
    Wqkv = np.asarray(Wqkv, dtype=np.float32)
    Wout = np.asarray(Wout, dtype=np.float32)

    xT = np.ascontiguousarray(x.reshape(TT, E).T).astype(npbf16)
    A, Bp, P = _rope_tables()
    ropeA = A.astype(np.float32)
    ropeBp = Bp.astype(np.float32)
    perm = P.astype(npbf16)
    triu = np.triu(np.ones((TKT, TKT), dtype=np.float32)).astype(npbf16)

    in_maps = []
    for c in range(NCORES):
        r0 = c * DD
        wq = np.ascontiguousarray(Wqkv[r0:r0 + DD, :].T).astype(npbf16)
        wk = np.ascontiguousarray(Wqkv[E + r0:E + r0 + DD, :].T).astype(npbf16)
        wv = np.ascontiguousarray(Wqkv[2 * E + r0:2 * E + r0 + DD, :].T).astype(npbf16)
        wo = np.ascontiguousarray(Wout[:, r0:r0 + DD].T).astype(npbf16)
        in_maps.append({
            "xT": xT, "wqT": wq, "wkT": wk, "wvT": wv, "woT": wo,
            "ropeA": ropeA, "ropeBp": ropeBp, "perm": perm, "triu": triu,
        })

    if "nc" not in _CACHE:
        _CACHE["nc"] = build_nc()
    res = run_bass_kernel_spmd(_CACHE["nc"], in_maps, core_ids=list(range(NCORES)))

    acc = np.zeros((TT, E), dtype=np.float32)
    for c in range(NCORES):
        acc += res.results[c]["out"]
    return acc.reshape(B, T, E)


# revision 21
# speedup vs baseline: 1.4424x; 1.4424x over previous
"""Distributed Trainium2 attention kernel (8 NeuronCores).

Problem: B=2, T=2048, E=1024, H=16 causal attention with partial RoPE
(first quarter of head_dim rotates, second quarter identity), fused QKV
projection and output projection.

Sharding: tensor-parallel over heads — each core owns 2 heads (dd=128 of
the E dims) for both batch elements. Each core computes q/k/v for its
heads from the full x, runs causal attention, and produces a partial
output projection (contraction over its 128 context dims). The host sums
the 8 partial outputs.

On-core layout choices (all "T" suffixes mean head-dim-on-partitions):
  - qT/kT [128, 4096]: rows = 2 heads x 64 dims, cols = flat b*2048+t.
  - scores computed transposed: S_T[tk, tq] = k_h^T-slice @ q_h, with the
    two heads packed into the 128x128 PE array as row groups (K=64 each).
  - softmax without max-subtraction (scores are O(few) for this data);
    P = exp(S_T/8) via ScalarE straight out of PSUM.
  - denominators: ctx matmul lhsT is [v_h | ones] (M=65); row 64 of the
    ctx PSUM accumulator is the softmax denominator.
  - normalization: reciprocal of row 64, broadcast across 64 partitions
    with a K=1 matmul against a ones row, then one DVE multiply.
  - RoPE: y = A .* q + swap(Bp .* q) where A/Bp are host-precomputed
    [128, 2048] tables and swap is a PE matmul by a permutation matrix.
  - causal: tk-tiles beyond the diagonal are skipped; diagonal tiles are
    computed with a column-sliced rhs (N = 512-128r) and masked with a
    single [128,128] triu mask.
"""

import numpy as np
import ml_dtypes

import concourse.bacc as bacc
import concourse.mybir as mybir
from concourse import tile
from concourse.bass_utils import run_bass_kernel_spmd

BF16 = mybir.dt.bfloat16
F32 = mybir.dt.float32
F32R = mybir.dt.float32r
npbf16 = ml_dtypes.bfloat16

B, T, E, H = 2, 2048, 1024, 16
NCORES = 8
D = E // H            # 64 head dim
HLOC = H // NCORES    # 2 heads per core
DD = HLOC * D         # 128 local context dims
TT = B * T            # 4096 flat tokens
NE = E // 128         # 8 contraction tiles for projections
NTB = TT // 512       # 8 t-blocks for qkv phase
TQB = 512             # tq block size
NJ = T // TQB         # 4 tq blocks per batch
TKT = 128             # tk tile size
NTK = T // TKT        # 16 tk tiles per batch
ROPE_BASE = 10000.0
SCALE = 1.0 / np.sqrt(D)

EXP = mybir.ActivationFunctionType.Exp


def build_nc(reps=1, loop_n=0, parts=('qkv','attn','out')):
    nc = bacc.Bacc(None, target_bir_lowering=False, debug=False)

    xT = nc.declare_dram_parameter("xT", [E, TT], BF16, isOutput=False)
    wqT = nc.declare_dram_parameter("wqT", [E, DD], BF16, isOutput=False)
    wkT = nc.declare_dram_parameter("wkT", [E, DD], BF16, isOutput=False)
    wvT = nc.declare_dram_parameter("wvT", [E, DD], BF16, isOutput=False)
    woT = nc.declare_dram_parameter("woT", [DD, E], BF16, isOutput=False)
    ropeA = nc.declare_dram_parameter("ropeA", [DD, T], F32, isOutput=False)
    ropeBp = nc.declare_dram_parameter("ropeBp", [DD, T], F32, isOutput=False)
    perm = nc.declare_dram_parameter("perm", [DD, DD], BF16, isOutput=False)
    maskneg = nc.declare_dram_parameter("maskneg", [TKT, TKT], BF16, isOutput=False)
    ident = nc.declare_dram_parameter("ident", [TKT, TKT], BF16, isOutput=False)
    out = nc.declare_dram_parameter("out", [TT, E], BF16, isOutput=True)

    with tile.TileContext(nc) as tc:
        with (
            tc.tile_pool(name="const", bufs=1) as const,
            tc.tile_pool(name="big", bufs=1) as big,
            tc.tile_pool(name="xt", bufs=18) as xtp,
            tc.tile_pool(name="rope_tmp", bufs=4) as rtp,
            tc.tile_pool(name="pbuf", bufs=16) as pbuf,
            tc.tile_pool(name="norm", bufs=4) as normp,
            tc.tile_pool(name="osb", bufs=2) as osbp,
        ):
            # ---- resident constants ----
            wq_sb = const.tile([128, NE * DD], BF16, tag="wq")
            wk_sb = const.tile([128, NE * DD], BF16, tag="wk")
            wv_sb = const.tile([128, NE * DD], BF16, tag="wv")
            for w_sb, w_dram in ((wq_sb, wqT), (wk_sb, wkT), (wv_sb, wvT)):
                nc.sync.dma_start(
                    out=w_sb[:].rearrange("p (e m) -> p e m", m=DD),
                    in_=w_dram[:].rearrange("(e p) m -> p e m", p=128))
            perm_sb = const.tile([128, 128], BF16, tag="perm")
            nc.sync.dma_start(out=perm_sb[:], in_=perm[:])
            ra_sb = const.tile([128, T], F32, tag="ra")
            rb_sb = const.tile([128, T], F32, tag="rb")
            nc.sync.dma_start(out=ra_sb[:], in_=ropeA[:])
            nc.sync.dma_start(out=rb_sb[:], in_=ropeBp[:])
            maskneg_sb = const.tile([128, 128], BF16, tag="maskneg")
            nc.sync.dma_start(out=maskneg_sb[:], in_=maskneg[:])
            ident_sb = const.tile([128, 128], BF16, tag="ident")
            nc.sync.dma_start(out=ident_sb[:], in_=ident[:])
            wo_sb = const.tile([128, E], BF16, tag="wo")
            nc.sync.dma_start(out=wo_sb[:], in_=woT[:])
            ones_f = const.tile([128, 64], F32, tag="ones_f")
            nc.vector.memset(ones_f[:], 1.0)
            ones_sb = const.tile([128, 64], F32R, tag="ones")
            nc.vector.tensor_copy(out=ones_sb[:], in_=ones_f[:])

            # ---- resident activations ----
            qrot = big.tile([128, TT], BF16, tag="qrot")
            krot = big.tile([128, TT], BF16, tag="krot")
            # 32 tiles of [v_h0(64) | 1 | v_h1(64) | 1]
            v_all = big.tile([128, 32 * 130], BF16, tag="vall")
            nc.vector.memset(v_all[:], 1.0)

            # ================= phase 1: qkv + rope =================
            import contextlib
            loop_ctx = tc.For_i(0, loop_n, 1) if loop_n else contextlib.nullcontext()
            with loop_ctx:
             for rep in range(reps):
              with (tc.tile_pool(name=f"psA{rep}", bufs=4, space="PSUM") as psA,
                  tc.tile_pool(name=f"psV{rep}", bufs=4, space="PSUM") as psV):
                xts_pair = None
                for tb in range(NTB):
                    tsl = slice(tb * 512, (tb + 1) * 512)
                    rsl = slice((tb % (NTB // B)) * 512,
                                ((tb % (NTB // B)) + 1) * 512)
                    if tb % 2 == 0:
                        # load 2 t-blocks worth of xT per e-tile in one DMA
                        xts_pair = []
                        for e in range(NE):
                            xt = xtp.tile([128, 1024], BF16, tag="xt",
                                          name=f"xt_{tb}_{e}")
                            nc.scalar.dma_start(out=xt[:],
                                          in_=xT[e * 128:(e + 1) * 128,
                                                 tb * 512:(tb + 2) * 512])
                            xts_pair.append(xt)
                    h0 = (tb % 2) * 512
                    for name, w_sb, dst in (("q", wq_sb, qrot), ("k", wk_sb, krot)):
                        ps = psA.tile([128, 512], F32, tag="pa")
                        for e in range(NE):
                            nc.tensor.matmul(ps[:], w_sb[:, e * DD:(e + 1) * DD],
                                             xts_pair[e][:, h0:h0 + 512],
                                             start=(e == 0),
                                             stop=(e == NE - 1))
                        # rope: z = Bp .* ps ; sw = perm^T z ; dst = A .* ps + sw
                        z = rtp.tile([128, 512], BF16, tag="z")
                        nc.vector.tensor_mul(out=z[:], in0=ps[:], in1=rb_sb[:, rsl])
                        sw = psA.tile([128, 512], F32, tag="pa")
                        nc.tensor.matmul(sw[:], perm_sb[:], z[:],
                                         start=True, stop=True)
                        y0 = rtp.tile([128, 512], F32, tag="y0")
                        nc.vector.tensor_mul(out=y0[:], in0=ps[:], in1=ra_sb[:, rsl])
                        nc.vector.tensor_add(out=dst[:, tsl], in0=y0[:], in1=sw[:])
                    # v for the 4 t-subtiles of this block
                    for s in range(4):
                        vi = tb * 4 + s
                        pv = psV.tile([128, 512], F32, tag="pv")
                        for e in range(NE):
                            nc.tensor.matmul(
                                pv[:, 0:DD],
                                xts_pair[e][:, h0 + s * 128:h0 + (s + 1) * 128],
                                wv_sb[:, e * DD:(e + 1) * DD],
                                start=(e == 0), stop=(e == NE - 1))
                        nc.vector.tensor_copy(out=v_all[:, vi * 130:vi * 130 + 64],
                                              in_=pv[:, 0:64])
                        nc.vector.tensor_copy(out=v_all[:, vi * 130 + 65:vi * 130 + 129],
                                              in_=pv[:, 64:128])

              # =============== phase 2: attention + out proj ===============
              if 'attn' not in parts:
                  continue
              with (
                tc.tile_pool(name=f"psS{rep}", bufs=2, space="PSUM") as psS,
                tc.tile_pool(name=f"psC{rep}", bufs=4, space="PSUM") as psC,
                tc.tile_pool(name=f"psO{rep}", bufs=2, space="PSUM") as psO,
              ):
                for j in range(NJ):
                    for b in range(B):
                        ni = 4 * j + 4  # tk tiles for this tq block
                        qsl = slice(b * T + j * TQB, b * T + (j + 1) * TQB)
                        pc = [psC.tile([128, 512], F32, tag="pc", name=f"pc_{b}_{j}_{h}")
                              for h in range(HLOC)]
                        for i in range(ni):
                            r = i - 4 * j  # >=0 on diagonal tiles
                            off = 128 * r if r > 0 else 0
                            n = 512 - off
                            qsl_i = slice(b * T + j * TQB + off, b * T + (j + 1) * TQB)
                            ksl = slice(b * T + i * TKT, b * T + (i + 1) * TKT)
                            vi = b * NTK + i
                            for h in range(HLOC):
                                hsl = slice(h * 64, (h + 1) * 64)
                                ps = psS.tile([128, 512], F32, tag="ps")
                                diag = r >= 0
                                nc.tensor.matmul(ps[:, 0:n], krot[hsl, ksl],
                                                 qrot[hsl, qsl_i],
                                                 start=True, stop=not diag)
                                if diag:
                                    # additive -1e9 on tk>tq of the leading
                                    # 128-col triangle, via PE accumulate
                                    nc.tensor.matmul(ps[:, 0:128], ident_sb[:],
                                                     maskneg_sb[:],
                                                     start=False, stop=True)
                                p_sb = pbuf.tile([128, 512], BF16, tag="p")
                                nc.scalar.activation(p_sb[:, off:512], ps[:, 0:n],
                                                     EXP, scale=float(SCALE))
                                nc.tensor.matmul(
                                    pc[h][0:65, off:512],
                                    v_all[:, vi * 130 + h * 65:vi * 130 + h * 65 + 65],
                                    p_sb[:, off:512],
                                    start=(i == 0), stop=(i == ni - 1))
                        # normalize: recip of denominators, broadcast, multiply
                        ctxn = normp.tile([128, 512], BF16, tag="ctxn")
                        bsb = normp.tile([128, 512], F32, tag="bsb")
                        for h in range(HLOC):
                            rc = normp.tile([65, 512], F32R, tag="recip")
                            with nc.allow_low_precision(reason="f32r recip for bcast"):
                                nc.vector.reciprocal(out=rc[64:65, :],
                                                     in_=pc[h][64:65, :])
                            pB = psO.tile([128, 512], F32, tag="po",
                                          name=f"pB_{b}_{j}_{h}")
                            nc.tensor.matmul(pB[0:64, :],
                                             ones_sb[64:65, :], rc[64:65, :],
                                             start=True, stop=True)
                            nc.vector.tensor_copy(out=bsb[h * 64:(h + 1) * 64, :],
                                                  in_=pB[0:64, :])
                        for h in range(HLOC):
                            nc.vector.tensor_mul(out=ctxn[h * 64:(h + 1) * 64, :],
                                                 in0=pc[h][0:64, :],
                                                 in1=bsb[h * 64:(h + 1) * 64, :])
                        # out projection: partial over this core's 128 dims
                        if 'out' not in parts:
                            continue
                        o_big = osbp.tile([128, 4096], BF16, tag="osb",
                                          name=f"obig_{b}_{j}")
                        for s in range(4):
                            for f in range(2):
                                po = psO.tile([128, 512], F32, tag="po")
                                nc.tensor.matmul(po[:],
                                                 ctxn[:, s * 128:(s + 1) * 128],
                                                 wo_sb[:, f * 512:(f + 1) * 512],
                                                 start=True, stop=True)
                                nc.vector.tensor_copy(
                                    out=o_big[:, s * 1024 + f * 512:
                                              s * 1024 + (f + 1) * 512],
                                    in_=po[:])
                        row0 = b * T + j * TQB
                        nc.sync.dma_start(
                            out=out[row0:row0 + TQB, :].rearrange(
                                "(s p) m -> p s m", p=128),
                            in_=o_big[:].rearrange("p (s m) -> p s m", m=E))
    nc.compile()
    return nc


def _rope_tables():
    quarter = D // 4  # 16
    inv = np.concatenate([
        ROPE_BASE ** (-np.arange(quarter, dtype=np.float64) / quarter),
        np.zeros(quarter),
    ])  # [32]
    t = np.arange(T, dtype=np.float64)
    theta = np.outer(t, inv)  # [T, 32]
    cos = np.cos(theta)
    sin = np.sin(theta)
    A = np.zeros((DD, T), dtype=np.float32)
    Bp = np.zeros((DD, T), dtype=np.float32)
    for dd in range(DD):
        dh = dd % D
        jj = dh % (D // 2)
        A[dd] = cos[:, jj]
        Bp[dd] = sin[:, jj] if dh < D // 2 else -sin[:, jj]
    P = np.zeros((DD, DD), dtype=np.float32)
    for k in range(DD):
        dh = k % D
        partner = k + D // 2 if dh < D // 2 else k - D // 2
        P[k, partner] = 1.0
    return A, Bp, P


_CACHE = {}


def kernel(x, Wqkv, Wout):
    x = np.asarray(x, dtype=np.float32)
    Wqkv = np.asarray(Wqkv, dtype=np.float32)
    Wout = np.asarray(Wout, dtype=np.float32)

    xT = np.ascontiguousarray(x.reshape(TT, E).T).astype(npbf16)
    A, Bp, P = _rope_tables()
    ropeA = A.astype(np.float32)
    ropeBp = Bp.astype(np.float32)
    perm = P.astype(npbf16)
    lowtri = np.tril(np.ones((TKT, TKT), dtype=np.float32), k=-1)
    maskneg = (lowtri * -1e9).astype(npbf16)
    ident = np.eye(TKT, dtype=np.float32).astype(npbf16)

    in_maps = []
    for c in range(NCORES):
        r0 = c * DD
        wq = np.ascontiguousarray(Wqkv[r0:r0 + DD, :].T).astype(npbf16)
        wk = np.ascontiguousarray(Wqkv[E + r0:E + r0 + DD, :].T).astype(npbf16)
        wv = np.ascontiguousarray(Wqkv[2 * E + r0:2 * E + r0 + DD, :].T).astype(npbf16)
        wo = np.ascontiguousarray(Wout[:, r0:r0 + DD].T).astype(npbf16)
        in_maps.append({
            "xT": xT, "wqT": wq, "wkT": wk, "wvT": wv, "woT": wo,
            "ropeA": ropeA, "ropeBp": ropeBp, "perm": perm,
            "maskneg": maskneg, "ident": ident,
        })

    if "nc" not in _CACHE:
        _CACHE["nc"] = build_nc()
    res = run_bass_kernel_spmd(_CACHE["nc"], in_maps, core_ids=list(range(NCORES)))

    acc = np.zeros((TT, E), dtype=np.float32)
    for c in range(NCORES):
        acc += res.results[c]["out"].astype(np.float32)
    return acc.reshape(B, T, E)
